# revision 16
# baseline (speedup 1.0000x reference)
"""Trainium2 Bass kernel for nn_AdaptiveMoodCoherenceHysteresis.

Math (after simplification of the reference):
  triad = diff + diff^T == 0  =>  triad_mag diagonal == 1, so
  plv[b,f]  = |mean_t exp(i*phases[b,f,t])|
  coh[b]    = clip(mean_f sqrt(plv[b,f] + 1e-12), 0, 1)
  v         = coh - prev_coh
  alpha     = prev_alpha + BETA*(AMIN + (AMAX-AMIN)*sigmoid(8|v|-1.5) - prev_alpha)
  out       = prev_coh + alpha*v          (== alpha*coh + (1-alpha)*prev_coh)

Sharding: pure data parallel, one batch element per NeuronCore (B=8, 8 cores).

Per-core kernel ([64, 2048] f32 shard, viewed as [128, 1024], p = h*64+f):
  DVE  : y = x/(2pi); k = rint(y) (magic const); d = y-k in [-.5,.5]; a = |d|
  ACT  : sin(2pi*d) and cos = sin(pi/2 - 2pi*a), each with accum_out giving
         per-partition sums directly (no big reductions)
  fold : h-halves summed via tiny DMA gather to one partition, then
         plv2 = (sum cos)^2 + (sum sin)^2 per f
  tail : coh = mean_f plv2^(1/4) / sqrt(T) via exp(0.25*ln(plv2) - 0.5*lnT),
         sigmoid via exp + DVE reciprocal, hysteresis lerp, DMA out [1].
"""
import numpy as np
import concourse.bass as bass
import concourse.mybir as mybir
from concourse.bass_utils import run_bass_kernel_spmd

F, T = 64, 2048
P = 128                      # partitions: (h, f), h in {0,1} halves of T
W = T // 2                   # free dim per partition = 1024
N_CORES = 8

ALPHA_MIN = 0.08
ALPHA_MAX = 0.45
BETA = 0.12
SIG_SLOPE = 8.0
SIG_OFFSET = 1.5

TWO_PI = 2.0 * np.pi
INV_2PI = 1.0 / TWO_PI
MAGIC = 1.5 * 2.0 ** 23      # rint(x) = (x + MAGIC) - MAGIC for |x| < 2^22
A = mybir.AluOpType
AF = mybir.ActivationFunctionType
F32 = mybir.dt.float32


def build(nchunk=2, repeat=1, debug=False):
    """Build the per-core Bass program. `repeat` re-runs the whole pipeline
    (for differential wall-clock timing); output is from the last iteration."""
    C = W // nchunk
    nc = bass.Bass()
    ph_in = nc.declare_dram_parameter("phases", [F, T], F32, isOutput=False)
    prev_in = nc.declare_dram_parameter("prev", [1, 2], F32, isOutput=False)
    out_d = nc.declare_dram_parameter("out", [1, 1], F32, isOutput=True)
    if debug:
        dbg_acc = nc.declare_dram_parameter("dbg_acc", [P, 2 * nchunk], F32,
                                            isOutput=True)
        dbg_row = nc.declare_dram_parameter("dbg_row", [1, 256], F32,
                                            isOutput=True)
        dbg_fold = nc.declare_dram_parameter("dbg_fold", [1, 128], F32,
                                             isOutput=True)
        dbg_plv2 = nc.declare_dram_parameter("dbg_plv2", [1, 64], F32,
                                             isOutput=True)
        dbg_ut = nc.declare_dram_parameter("dbg_ut", [1, 64], F32,
                                           isOutput=True)
        dbg_sc = nc.declare_dram_parameter("dbg_sc", [1, 16], F32,
                                           isOutput=True)
        dbg_x0 = nc.declare_dram_parameter("dbg_x0", [P, W // nchunk], F32,
                                           isOutput=True)
        dbg_d0 = nc.declare_dram_parameter("dbg_d0", [P, W // nchunk], F32,
                                           isOutput=True)
        dbg_d1 = nc.declare_dram_parameter("dbg_d1", [P, W // nchunk], F32,
                                           isOutput=True)
        dbg_a0 = nc.declare_dram_parameter("dbg_a0", [P, W // nchunk], F32,
                                           isOutput=True)

    # [2, 64, 1024] view with (h, f, t) iteration order == [128, 1024] p=(h f)
    ph = ph_in[:].rearrange("f (h t) -> f h t", h=2).rearrange("f h t -> h f t")

    from contextlib import ExitStack
    with ExitStack() as stack:
        def sb(name, shape):
            return stack.enter_context(nc.sbuf_tensor(name, shape, F32))

        x0, x1 = sb("x0", [P, C]), sb("x1", [P, C])
        yt, kt = sb("yt", [P, C]), sb("kt", [P, C])
        d0, d1 = sb("d0", [P, C]), sb("d1", [P, C])
        a0, a1 = sb("a0", [P, C]), sb("a1", [P, C])
        scr = sb("scr", [P, C])
        acc = sb("acc", [P, 2 * nchunk])
        cs2 = sb("cs2", [P, 2])
        row = sb("row", [1, 256])
        fold = sb("fold", [1, 128])
        sq = sb("sq", [1, 128])
        plv2 = sb("plv2", [1, 64])
        lnv = sb("lnv", [1, 64])
        ut = sb("ut", [1, 64])
        prev_t = sb("prev_t", [1, 2])
        sc = sb("sc", [1, 16])    # small scalars, one per col
        halfpi = sb("halfpi", [P, 1])
        bexp = sb("bexp", [1, 1])
        bsig = sb("bsig", [1, 1])
        ch_sem0 = stack.enter_context(nc.semaphore("ch_sem0"))
        ch_sem1 = stack.enter_context(nc.semaphore("ch_sem1"))
        pv_sem = stack.enter_context(nc.semaphore("pv_sem"))
        g_sem = stack.enter_context(nc.semaphore("g_sem"))
        v_sem = stack.enter_context(nc.semaphore("v_sem"))
        act_sem = stack.enter_context(nc.semaphore("act_sem"))
        ch_sems = [ch_sem0, ch_sem1]
        block = stack.enter_context(nc.Block())
        xb = [x0, x1]
        db = [d0, d1]
        ab = [a0, a1]
        # sc columns: 0=S,1=coh,2=v,3=av,4=e,5=w,6=g,7=t1,8=t2,9=alpha,10=m,11=out
        S_, COH, V_, AV, E_, WW, G_, T1, T2, ALP, M_, OUT = range(12)

        # per-iteration semaphore deltas
        V_PER = nchunk + 4
        ACT_PER = nchunk + 2
        GATHER_PER = (2 + 10) if debug else 2   # gather + out (+ debug dumps)

        @block.sync
        def _(sync):
            for r in range(repeat):
                vb = r * V_PER
                sync.dma_start(prev_t[:], prev_in[:]).then_inc(pv_sem, 16)
                for i in range(nchunk):
                    g = r * nchunk + i
                    if g >= 2:
                        # buffer reuse: chunk g-2's DVE must be done
                        pr, pi = divmod(g - 2, nchunk)
                        sync.wait_ge(v_sem, pr * V_PER + pi + 1)
                    sync.dma_start(
                        xb[g % 2][:], ph[:, :, i * C:(i + 1) * C]
                    ).then_inc(ch_sems[g % 2], 16)
                sync.wait_ge(v_sem, vb + nchunk + 1)
                sync.dma_start(
                    row[0:1, 0:256].rearrange("one (p c) -> one p c", c=2),
                    cs2[:, 0:2],
                ).then_inc(g_sem, 16)
                sync.wait_ge(v_sem, vb + nchunk + 4)
                sync.dma_start(out_d[:], sc[0:1, OUT:OUT + 1]).then_inc(g_sem, 16)
                if debug:
                    sync.dma_start(dbg_acc[:], acc[:]).then_inc(g_sem, 16)
                    sync.dma_start(dbg_row[:], row[:]).then_inc(g_sem, 16)
                    sync.dma_start(dbg_fold[:], fold[:]).then_inc(g_sem, 16)
                    sync.dma_start(dbg_plv2[:], plv2[:]).then_inc(g_sem, 16)
                    sync.dma_start(dbg_ut[:], ut[:]).then_inc(g_sem, 16)
                    sync.dma_start(dbg_sc[:], sc[:]).then_inc(g_sem, 16)
                    sync.dma_start(dbg_x0[:], x0[:]).then_inc(g_sem, 16)
                    sync.dma_start(dbg_d0[:], d0[:]).then_inc(g_sem, 16)
                    sync.dma_start(dbg_d1[:], d1[:]).then_inc(g_sem, 16)
                    sync.dma_start(dbg_a0[:], a0[:]).then_inc(g_sem, 16)

        @block.vector
        def _(vector):
            vector.memset(halfpi[:], float(np.pi / 2))
            vector.memset(bexp[:], float(-0.5 * np.log(T)))
            vector.memset(bsig[:], float(SIG_OFFSET))
            for r in range(repeat):
                vb = r * V_PER
                actb = r * ACT_PER
                for i in range(nchunk):
                    g = r * nchunk + i
                    vector.wait_ge(ch_sems[g % 2], 16 * (g // 2 + 1))
                    x = xb[g % 2]
                    vector.tensor_scalar_mul(yt[:], x[:], INV_2PI)
                    vector.tensor_scalar(kt[:], yt[:], MAGIC, MAGIC, A.add,
                                         A.subtract)
                    vector.tensor_sub(db[g % 2][:], yt[:], kt[:])
                    vector.scalar_tensor_tensor(
                        ab[g % 2][:], db[g % 2][:], -1.0, db[g % 2][:],
                        A.mult, A.max,
                    ).then_inc(v_sem, 1)
                # combine chunk accums: cs2[:,0] = sum cos, cs2[:,1] = sum sin
                vector.wait_ge(act_sem, actb + nchunk)
                if nchunk == 1:
                    vector.tensor_scalar_mul(cs2[:, 0:2], acc[:, 0:2], 1.0)
                elif nchunk == 2:
                    vector.tensor_add(cs2[:, 0:2], acc[:, 0:4:2], acc[:, 1:4:2])
                else:
                    vector.tensor_reduce(
                        cs2[:, 0:2],
                        acc[:, 0:2 * nchunk].rearrange("p (c i) -> p c i", c=2),
                        axis=mybir.AxisListType.X, op=A.add,
                    )
                vector.sem_inc(v_sem, 1)  # -> vb + nchunk + 1
                # fold h halves + plv2, on partition 0
                vector.wait_ge(g_sem, 16 * r * GATHER_PER + 16)
                # row[0, 2*(h*64+f) + c]; fold over h -> fold[0, c*64+f]
                rr4 = row[0:1, 0:256].rearrange("one (h f c) -> one h f c",
                                                h=2, c=2)
                vector.tensor_add(
                    fold[0:1, 0:128].rearrange("one (c f) -> one c f", c=2),
                    rr4[:, 0].rearrange("one f c -> one c f"),
                    rr4[:, 1].rearrange("one f c -> one c f"),
                )
                vector.drain()
                vector.tensor_mul(sq[:], fold[:], fold[:])
                vector.drain()
                vector.tensor_add(plv2[0:1, 0:64], sq[0:1, 0:64], sq[0:1, 64:128])
                vector.sem_inc(v_sem, 1)  # -> vb + nchunk + 2
                # coh / v / |v|
                vector.wait_ge(act_sem, actb + nchunk + 1)
                vector.tensor_scalar(sc[0:1, COH:COH + 1], sc[0:1, S_:S_ + 1],
                                     1.0 / F, 1.0, A.mult, A.min)
                vector.wait_ge(pv_sem, 16 * (r + 1))
                vector.drain()
                vector.tensor_sub(sc[0:1, V_:V_ + 1], sc[0:1, COH:COH + 1],
                                  prev_t[0:1, 0:1])
                vector.drain()
                vector.scalar_tensor_tensor(
                    sc[0:1, AV:AV + 1], sc[0:1, V_:V_ + 1], -1.0,
                    sc[0:1, V_:V_ + 1], A.mult, A.max,
                ).then_inc(v_sem, 1)      # -> vb + nchunk + 3
                # sigmoid + lerp
                vector.wait_ge(act_sem, actb + nchunk + 2)
                vector.tensor_scalar_add(sc[0:1, WW:WW + 1], sc[0:1, E_:E_ + 1],
                                         1.0)
                vector.drain()
                vector.drain()
                vector.reciprocal(sc[0:1, G_:G_ + 1], sc[0:1, WW:WW + 1])
                vector.drain()
                vector.tensor_scalar(sc[0:1, T1:T1 + 1], sc[0:1, G_:G_ + 1],
                                     BETA * (ALPHA_MAX - ALPHA_MIN),
                                     BETA * ALPHA_MIN, A.mult, A.add)
                vector.tensor_scalar_mul(sc[0:1, T2:T2 + 1], prev_t[0:1, 1:2],
                                         1.0 - BETA)
                vector.drain()
                vector.tensor_add(sc[0:1, ALP:ALP + 1], sc[0:1, T1:T1 + 1],
                                  sc[0:1, T2:T2 + 1])
                vector.drain()
                vector.tensor_mul(sc[0:1, M_:M_ + 1], sc[0:1, ALP:ALP + 1],
                                  sc[0:1, V_:V_ + 1])
                vector.drain()
                vector.tensor_add(sc[0:1, OUT:OUT + 1], sc[0:1, M_:M_ + 1],
                                  prev_t[0:1, 0:1]).then_inc(v_sem, 1)
                # -> vb + nchunk + 4

        @block.scalar
        def _(scalar):
            for r in range(repeat):
                vb = r * V_PER
                for i in range(nchunk):
                    g = r * nchunk + i
                    scalar.wait_ge(v_sem, vb + i + 1)
                    scalar.activation(scr[:], db[g % 2][:], AF.Sin,
                                      bias=0.0, scale=TWO_PI,
                                      accum_out=acc[:, i:i + 1])
                    scalar.activation(scr[:], ab[g % 2][:], AF.Sin,
                                      bias=halfpi[:], scale=-TWO_PI,
                                      accum_out=acc[:, nchunk + i:nchunk + i + 1]
                                      ).then_inc(act_sem, 1)
                # ln/exp tail: coh pieces
                scalar.wait_ge(v_sem, vb + nchunk + 2)
                scalar.activation(lnv[:], plv2[:], AF.Ln, bias=0.0, scale=1.0)
                scalar.activation(ut[:], lnv[:], AF.Exp, bias=bexp[:],
                                  scale=0.25,
                                  accum_out=sc[0:1, S_:S_ + 1]
                                  ).then_inc(act_sem, 1)
                scalar.wait_ge(v_sem, vb + nchunk + 3)
                scalar.activation(sc[0:1, E_:E_ + 1], sc[0:1, AV:AV + 1],
                                  AF.Exp, bias=bsig[:], scale=-SIG_SLOPE
                                  ).then_inc(act_sem, 1)

    return nc


_cache = {}


def _get_nc(nchunk=2, repeat=1):
    key = (nchunk, repeat)
    if key not in _cache:
        _cache[key] = build(nchunk=nchunk, repeat=repeat)
    return _cache[key]


def kernel(phases, prev_coh, prev_alpha):
    phases = np.ascontiguousarray(np.asarray(phases, dtype=np.float32))
    prev_coh = np.asarray(prev_coh, dtype=np.float32)
    prev_alpha = np.asarray(prev_alpha, dtype=np.float32)
    B = phases.shape[0]
    assert B == N_CORES and phases.shape[1:] == (F, T)

    nc = _get_nc()
    in_maps = [
        {
            "phases": phases[b],
            "prev": np.array([[prev_coh[b], prev_alpha[b]]], dtype=np.float32),
        }
        for b in range(B)
    ]
    res = run_bass_kernel_spmd(nc, in_maps, core_ids=list(range(N_CORES))).results
    return np.array([res[b]["out"][0, 0] for b in range(B)], dtype=np.float32)
